# revision 25
# baseline (speedup 1.0000x reference)
"""Trainium2 Bass kernel for nn_CNN2D_37873021616665 (dense_cnn).

Data-parallel over batch: 16 samples -> 8 cores x 2 samples. Per core:
  1. 32x32 mean-pool on the PE: host layout xs5[b,h,s,c,j] (w = 32j+s) makes
     every moving operand a contiguous [h, (c j)] slab; 64 matmuls accumulate
     straight into ctx [7i, (c j)] in one PSUM bank per sample.
  2. asymmetric DMA rings: SP carries only b0's h<128 half, ACT the rest, so
     b0's context is ready ~3x earlier and its whole roi-head -> box ->
     gather -> interp chain hides under b1's pooling stream (HWDGE rings are
     FIFO, so b0's gathers would otherwise queue behind the stream).
  3. per-sample roi/box path: b1's ctx lands on partitions 64:127 and uses
     the second copy of the block-diagonal weights; boxes do not depend on
     the attention branch at all.
  4. attention head batched over samples on (b, c) partitions; LN/softmax/aw
     (with the torch-.view regroup via a DRAM round-trip) overlaps gathers.
  5. bilinear combine with clamp/validity/0.25-subsample folded into
     weights; final 3x3 conv batched over samples (36 bf16 matmuls, N=98),
     combined with the attention-derived scalars by linearity.
"""
import numpy as np

B = 16
NCORES = 8
BL = B // NCORES          # samples per core
C = 64
H = W = 224
POOL = 7
NK = 4
TEMP = 5.0

_CACHED = {}


# --------------------------------------------------------------------------
# host-side constant packing (pure layout; replicated to all cores)
# --------------------------------------------------------------------------
def make_consts(inputs):
    import ml_dtypes
    f32 = np.float32
    bf16 = ml_dtypes.bfloat16
    fc1_w = np.asarray(inputs["fc1_w"], f32)
    fc2_w = np.asarray(inputs["fc2_w"], f32)
    fc2_b = np.asarray(inputs["fc2_b"], f32)
    ln_g = np.asarray(inputs["ln_g"], f32)
    ln_b = np.asarray(inputs["ln_b"], f32)
    roi_w1 = np.asarray(inputs["roi_w1"], f32)
    roi_w2 = np.asarray(inputs["roi_w2"], f32)
    roi_b2 = np.asarray(inputs["roi_b2"], f32)
    weight = np.asarray(inputs["weight"], f32)
    bias = np.asarray(inputs["bias"], f32)

    k128 = np.zeros((128, 7), f32)
    for h in range(128):
        k128[h, h // 32] = 1.0 / 1024.0
    k96 = np.zeros((96, 7), f32)
    for h in range(96):
        k96[h, 4 + h // 32] = 1.0 / 1024.0
    k128 = k128.astype(bf16)        # 1/1024 is exact in bf16
    k96 = k96.astype(bf16)

    def blk(m):  # [p, q] -> [2p, 2q] block diagonal
        p, q = m.shape
        out = np.zeros((2 * p, 2 * q), f32)
        out[:p, :q] = m
        out[p:, q:] = m
        return out

    # kbd [128, 2624] bf16: fc1 blkdiag | roi1 blkdiag x9 | conv-w [64,64] x36
    kbd = np.zeros((128, 2624), f32)
    kbd[:, 0:32] = blk(fc1_w[:, :, 0, 0].T)
    for d in range(9):
        dy, dx = d // 3, d % 3
        kbd[:, 32 + d * 32:32 + (d + 1) * 32] = blk(roi_w1[:, :, dy, dx].T)
    for k in range(4):
        for d in range(9):
            dy, dx = d // 3, d % 3
            kbd[0:64, 320 + (k * 9 + d) * 64:320 + (k * 9 + d + 1) * 64] = \
                weight[k, :, :, dy, dx].T
    kbd = kbd.astype(bf16)

    # k32 [32, 80] bf16: fc2 blkdiag x9 | roi_w2/49 (single copy, rows 0:16)
    k32 = np.zeros((32, 80), f32)
    for d in range(9):
        dy, dx = d // 3, d % 3
        k32[:, d * 8:(d + 1) * 8] = blk(fc2_w[:, :, dy, dx].T)
    k32[0:16, 72:76] = (roi_w2[:4, :, 0, 0] / 49.0).T
    k32 = k32.astype(bf16)

    k49 = np.zeros((49, 33), f32)
    k49[:, 0:8] = np.tile(ln_g, (49, 2))
    k49[:, 8:16] = np.tile(ln_b, (49, 2))
    t = np.arange(196).reshape(49, 4)
    for s in range(4):
        k49[:, 16 + s * 4:16 + (s + 1) * 4] = ((t >= 49 * s) & (t < 49 * (s + 1)))
    k49[:, 32] = 1.0

    kbt = np.tile(bias.T, (2, 1)).astype(f32)     # [128, 4] (only 0:64 used)

    k8 = np.zeros((8, 2), f32)
    k8[:, 0] = np.tile(fc2_b, 2)
    k8[:, 1] = np.tile(roi_b2[:4] * 224.0, 2)

    k7 = np.eye(8, dtype=f32)

    k1 = np.zeros((1, 142), f32)
    off = (np.arange(7, dtype=f32)[:, None] + (np.arange(2, dtype=f32)[None, :] + 0.5) / 2)
    k1[0, 0:14] = off.reshape(-1)
    k1[0, 14:142] = 1.0

    return {"k128": k128, "k96": k96, "kbd": kbd, "k32": k32, "k49": k49,
            "kbt": kbt, "k8": k8, "k7": k7, "k1": k1}


# --------------------------------------------------------------------------
# device program
# --------------------------------------------------------------------------
def build_nc(repeat=1):
    import concourse.bass as bass
    import concourse.bacc as bacc
    import concourse.mybir as mybir
    import concourse.tile as tile
    from concourse.bass import ds

    f32 = mybir.dt.float32
    i32 = mybir.dt.int32
    bf16 = mybir.dt.bfloat16
    Alu = mybir.AluOpType
    Act = mybir.ActivationFunctionType
    ET = mybir.EngineType

    nc = bacc.Bacc("TRN2", target_bir_lowering=False)

    xs5_h = nc.dram_tensor("xs5", [BL, H, 32, C, 7], bf16, kind="ExternalInput")
    xg_h = nc.dram_tensor("xg", [BL, C, H, W], bf16, kind="ExternalInput")
    k128_h = nc.dram_tensor("k128", [128, 7], bf16, kind="ExternalInput")
    k96_h = nc.dram_tensor("k96", [96, 7], bf16, kind="ExternalInput")
    kbd_h = nc.dram_tensor("kbd", [128, 2624], bf16, kind="ExternalInput")
    k32_h = nc.dram_tensor("k32", [32, 80], bf16, kind="ExternalInput")
    k49_h = nc.dram_tensor("k49", [49, 33], f32, kind="ExternalInput")
    kbt_h = nc.dram_tensor("kbt", [128, 4], f32, kind="ExternalInput")
    k8_h = nc.dram_tensor("k8", [8, 2], f32, kind="ExternalInput")
    k7_h = nc.dram_tensor("k7", [8, 8], f32, kind="ExternalInput")
    k1_h = nc.dram_tensor("k1", [1, 142], f32, kind="ExternalInput")
    out_h = nc.dram_tensor("out", [BL, C, POOL, POOL], f32, kind="ExternalOutput")

    xg_flat = xg_h[:].rearrange("b c h w -> b c (h w)")

    from contextlib import ExitStack

    with tile.TileContext(nc) as tc, ExitStack() as est:
        cp = est.enter_context(tc.tile_pool(name="const", bufs=1))
        xp = est.enter_context(tc.tile_pool(name="xtiles", bufs=4))
        wp = est.enter_context(tc.tile_pool(name="work", bufs=2))
        gp = est.enter_context(tc.tile_pool(name="gather", bufs=1))
        pp_acc = est.enter_context(tc.tile_pool(name="ps_acc", bufs=2, space="PSUM"))
        pp_m = est.enter_context(tc.tile_pool(name="ps_misc", bufs=2, space="PSUM"))
        pp_c = est.enter_context(tc.tile_pool(name="ps_conv", bufs=1, space="PSUM"))
        dp = est.enter_context(tc.tile_pool(name="dscratch", bufs=1, space="DRAM"))

        # ---- load constants (gpsimd ring: keeps SP/ACT free for x) ----
        kc = {}
        for name, hdl, shp, dt_ in (("k128", k128_h, [128, 7], bf16),
                                    ("k96", k96_h, [96, 7], bf16),
                                    ("kbd", kbd_h, [128, 2624], bf16),
                                    ("k32", k32_h, [32, 80], bf16),
                                    ("k49", k49_h, [49, 33], f32),
                                    ("kbt", kbt_h, [128, 4], f32),
                                    ("k8", k8_h, [8, 2], f32),
                                    ("k7", k7_h, [8, 8], f32),
                                    ("k1", k1_h, [1, 142], f32)):
            t = cp.tile(shp, dt_, tag=name)
            nc.gpsimd.dma_start(out=t[:], in_=hdl[:])
            kc[name] = t
        fc1l = kc["kbd"][:, 0:32]
        # roi1 block for sample b: b0 -> top-left, b1 -> bottom-right copy
        roi1l = lambda b, d: kc["kbd"][b * 64:(b + 1) * 64,
                                       32 + d * 32 + b * 16:32 + d * 32 + b * 16 + 16]
        wl = lambda k, d: kc["kbd"][0:64, 320 + (k * 9 + d) * 64:320 + (k * 9 + d + 1) * 64]
        fc2l = lambda d: kc["k32"][:, d * 8:(d + 1) * 8]
        w2l = kc["k32"][0:16, 72:76]
        lng8 = kc["k49"][:, 0:8]
        lnb8 = kc["k49"][:, 8:16]
        m4 = lambda s: kc["k49"][:, 16 + s * 4:16 + (s + 1) * 4]
        ones49 = kc["k49"][:, 32:33]
        biasT = kc["kbt"][:, :]
        fc2b = kc["k8"][:, 0:1]
        b224 = lambda b: kc["k8"][0:4, 1:2]
        id8 = kc["k7"][:]
        id7 = kc["k7"][0:7, 0:7]
        id4 = kc["k7"][0:4, 0:4]
        offv = kc["k1"][:, 0:14]
        ones1x = kc["k1"][:, 14:142]          # [1, 128] of ones

        for _rep in range(repeat):
            attflat = dp.tile([BL, 196], f32)
            NCK = 4

            # ---- issue every pooling chunk DMA up front (queues are
            # per-engine): SP ring = b0 h<128 only, ACT ring = the rest, so
            # SP drains early and b0's gathers aren't stuck behind the stream
            tiles = {}
            for b in range(BL):
                eng_a = nc.sync if b == 0 else nc.scalar
                for ck in range(NCK):
                    ta = xp.tile([128, 8, 448], bf16, tag=f"TA{b}", name=f"TA{b}_{ck}")
                    tb = xp.tile([96, 8, 448], bf16, tag=f"TB{b}", name=f"TB{b}_{ck}")
                    eng_a.dma_start(
                        out=ta[:],
                        in_=xs5_h[b, 0:128, ck * 8:(ck + 1) * 8]
                        .rearrange("p s c j -> p s (c j)"))
                    nc.scalar.dma_start(
                        out=tb[:],
                        in_=xs5_h[b, 128:224, ck * 8:(ck + 1) * 8]
                        .rearrange("p s c j -> p s (c j)"))
                    tiles[(b, ck)] = (ta, tb)

            # ---- PE warm-up: ~20 throwaway matmuls on the const tile so the
            # HAM clock gate is at 8/8 by the time the first chunk lands
            ps_warm = pp_m.tile([7, 448], f32, tag="psm")
            for w_i in range(20):
                nc.tensor.matmul(ps_warm[:], kc["k128"][:],
                                 kc["kbd"][:, w_i * 64:w_i * 64 + 448],
                                 start=True, stop=True)

            # shared tiles for the per-sample chains
            ps_ctxT = pp_m.tile([128, 7, 7], f32, tag="psT")
            ctx9 = wp.tile([128, 9, 9], bf16, tag="ctx9")
            nc.gpsimd.memset(ctx9[:], 0.0)
            z9b = wp.tile([64, BL, 9, 9], bf16, tag="z9b")
            nc.gpsimd.memset(z9b[:], 0.0)
            ps_pool_l = []
            wball_l = []

            # ========== per-sample: pool -> ctx -> roi -> box -> gather ====
            # (b0's whole chain is issued BEFORE b1's pooling matmuls: the PE
            # queue is strict FIFO, so anything issued after b1's pool MMs
            # would stall behind the DMA-paced stream)
            for b in range(BL):
                ps_pool = pp_acc.tile([7, 448], f32, tag="poolacc")
                for ck in range(NCK):
                    ta, tb = tiles[(b, ck)]
                    for sl in range(8):
                        nc.tensor.matmul(ps_pool[:], kc["k128"][:], ta[:, sl, :],
                                         start=(ck == 0 and sl == 0), stop=False)
                    for sl in range(8):
                        nc.tensor.matmul(ps_pool[:], kc["k96"][:], tb[:, sl, :],
                                         start=False, stop=(ck == NCK - 1 and sl == 7))
                ps_pool_l.append(ps_pool)
                pl, ph = b * 64, (b + 1) * 64
                ctx_sbb = wp.tile([7, 64, 7], f32, tag=f"ctx{b}", name=f"ctx{b}")
                nc.vector.tensor_copy(ctx_sbb[:], ps_pool[:].rearrange(
                    "p (c j) -> p c j", j=7))
                for j in range(7):
                    # plain matmul against the identity == transpose, but is
                    # allowed to land on PSUM base-partition 64 (b1's half)
                    nc.tensor.matmul(ps_ctxT[pl:ph, :, j], ctx_sbb[:, :, j], id7,
                                     start=True, stop=True)
                nc.vector.tensor_copy(ctx9[pl:ph, 1:8, 1:8], ps_ctxT[pl:ph])

                # roi head (second blkdiag copy for b1; all outputs at base 0)
                ps_r = pp_m.tile([16, 7, 7], f32, tag="psm")
                for d in range(9):
                    dy, dx = d // 3, d % 3
                    nc.tensor.matmul(ps_r[:], roi1l(b, d),
                                     ctx9[pl:ph, dy:dy + 7, dx:dx + 7],
                                     start=(d == 0), stop=(d == 8))
                r_sb = wp.tile([16, 7, 7], f32, tag=f"r_sb{b}", name=f"r_sb{b}")
                rsum = wp.tile([16, 1], f32, tag=f"rsum{b}", name=f"rsum{b}")
                nc.scalar.activation(r_sb[:], ps_r[:], Act.Relu, accum_out=rsum[:])
                rsum_b = wp.tile([16, 1], bf16, tag=f"rsumb{b}", name=f"rsumb{b}")
                nc.vector.tensor_copy(rsum_b[:], rsum[:])
                ps_bb = pp_m.tile([4, 1], f32, tag="psm")
                nc.tensor.matmul(ps_bb[:], w2l, rsum_b[:], start=True, stop=True)
                bb_sb = wp.tile([4, 1], f32, tag=f"bb{b}", name=f"bb{b}")
                nc.scalar.activation(bb_sb[:], ps_bb[:], Act.Identity,
                                     bias=b224(b), scale=224.0)
                ps_bx = pp_m.tile([1, 4], f32, tag="psm")
                nc.tensor.transpose(ps_bx[:], bb_sb[:], id4)
                bx = wp.tile([1, 4], f32, tag=f"bx{b}", name=f"bx{b}")
                nc.vector.tensor_copy(bx[:], ps_bx[:])

                # box math [1, {x,y}, 14]
                d2 = wp.tile([1, 2], f32, tag=f"d2{b}", name=f"d2{b}")
                nc.vector.tensor_sub(d2[:], bx[:, 2:4], bx[:, 0:2])
                nc.vector.tensor_scalar_max(d2[:], d2[:], 1.0)
                nc.vector.tensor_scalar_mul(d2[:], d2[:], 1.0 / 7.0)
                cs = wp.tile([1, 2, 14], f32, tag=f"cs{b}", name=f"cs{b}")
                for ax in range(2):
                    nc.vector.scalar_tensor_tensor(
                        cs[:, ax], offv, d2[:, ax:ax + 1],
                        bx[:, ax:ax + 1].broadcast_to([1, 14]),
                        op0=Alu.mult, op1=Alu.add)
                va = wp.tile([1, 2, 14], f32, tag=f"va{b}", name=f"va{b}")
                vb = wp.tile([1, 2, 14], f32, tag=f"vb{b}", name=f"vb{b}")
                nc.vector.tensor_scalar(va[:], cs[:], -1.0, None, op0=Alu.is_ge)
                nc.vector.tensor_scalar(vb[:], cs[:], 224.0, None, op0=Alu.is_le)
                nc.vector.tensor_mul(va[:], va[:], vb[:])
                # fold the 2x2-subsample 1/4 into validity (1/2 per axis)
                nc.vector.tensor_scalar_mul(va[:], va[:], 0.5)
                cc = wp.tile([1, 2, 14], f32, tag=f"cc{b}", name=f"cc{b}")
                nc.vector.tensor_scalar(cc[:], cs[:], 0.0, 223.0, op0=Alu.max, op1=Alu.min)
                iraw = wp.tile([1, 2, 14], i32, tag=f"iraw{b}", name=f"iraw{b}")
                nc.vector.tensor_copy(iraw[:], cc[:])
                c0 = wp.tile([1, 2, 14], f32, tag=f"c0{b}", name=f"c0{b}")
                nc.vector.tensor_copy(c0[:], iraw[:])
                cgt = wp.tile([1, 2, 14], f32, tag=f"cgt{b}", name=f"cgt{b}")
                nc.vector.tensor_tensor(cgt[:], c0[:], cc[:], op=Alu.is_gt)
                nc.vector.tensor_sub(c0[:], c0[:], cgt[:])
                nc.vector.tensor_scalar_min(c0[:], c0[:], 222.0)
                fr = wp.tile([1, 2, 14], f32, tag=f"fr{b}", name=f"fr{b}")
                nc.vector.tensor_sub(fr[:], cc[:], c0[:])
                tw = wp.tile([1, 2, 14], f32, tag=f"tw{b}", name=f"tw{b}")
                nc.vector.tensor_scalar(tw[:], fr[:], -1.0, 1.0, op0=Alu.mult, op1=Alu.add)
                wb_sb = wp.tile([1, 2, 14, 2], f32, tag=f"wbs{b}", name=f"wbs{b}")
                nc.vector.tensor_mul(wb_sb[:, :, :, 0], tw[:], va[:])
                nc.vector.tensor_mul(wb_sb[:, :, :, 1], fr[:], va[:])
                ioff = wp.tile([1, 2, 14], f32, tag=f"ioff{b}", name=f"ioff{b}")
                nc.vector.tensor_copy(ioff[:, 0], c0[:, 0])
                nc.vector.tensor_scalar_mul(ioff[:, 1], c0[:, 1], 224.0)
                ioffi = wp.tile([1, 2, 14], i32, tag=f"ioffi{b}", name=f"ioffi{b}")
                nc.vector.tensor_copy(ioffi[:], ioff[:])

                ps_wb = pp_m.tile([64, 56], f32, tag="psm")
                nc.tensor.matmul(ps_wb[:], ones1x[:, 0:64],
                                 wb_sb[:].rearrange("p t j a -> p (t j a)"),
                                 start=True, stop=True)
                wball = wp.tile([64, 2, 14, 2], f32, tag=f"wball{b}", name=f"wball{b}")
                nc.scalar.activation(wball[:].rearrange("p t j a -> p (t j a)"),
                                     ps_wb[:], Act.Identity)
                wball_l.append(wball)

                # gathers: both samples on the SP ring (idle once b0's half is in)
                _, yvals = nc.values_load_multi_w_load_instructions(
                    ioffi[:, 1, :], engines=[ET.SP],
                    min_val=0, max_val=49728, skip_runtime_bounds_check=True)
                g2 = gp.tile([64, 14, 2, 224], bf16, tag=f"g2{b}", name=f"g2{b}")
                for i in range(14):
                    nc.sync.dma_start(out=g2[:, i], in_=xg_flat[b, :, ds(yvals[i], 448)]
                                      .rearrange("c (r w) -> c r w", r=2))
                _, xvals = nc.values_load_multi_w_load_instructions(
                    ioffi[:, 0, :], engines=[ET.DVE],
                    min_val=0, max_val=222, skip_runtime_bounds_check=True)
                # x-interp: per-j pair * (wA, wB)
                m1 = wp.tile([64, 14, 2, 14, 2], f32, tag=f"m1{b}", name=f"m1{b}")
                for j in range(14):
                    nc.vector.tensor_tensor(
                        m1[:, :, :, j, :], g2[:, :, :, ds(xvals[j], 2)],
                        wball[:, 0, j].unsqueeze(1).unsqueeze(1)
                        .broadcast_to([64, 14, 2, 2]), op=Alu.mult)
                zc4 = wp.tile([64, 14, 2, 14], f32, tag=f"zc4{b}", name=f"zc4{b}")
                nc.vector.tensor_tensor(zc4[:], m1[:, :, :, :, 0], m1[:, :, :, :, 1],
                                        op=Alu.add)
                m2 = wp.tile([64, 14, 2, 14], f32, tag=f"m2{b}", name=f"m2{b}")
                nc.vector.tensor_tensor(m2[:], zc4[:],
                                        wball[:, 1].unsqueeze(3)
                                        .broadcast_to([64, 14, 2, 14]), op=Alu.mult)
                zr = wp.tile([64, 14, 14], f32, tag=f"zr{b}", name=f"zr{b}")
                nc.vector.tensor_tensor(zr[:], m2[:, :, 0, :], m2[:, :, 1, :], op=Alu.add)
                zr4 = zr[:].rearrange("p (i a) (j e) -> p i a j e", a=2, e=2)
                t1 = wp.tile([64, 7, 7], f32, tag=f"t1{b}", name=f"t1{b}")
                t2 = wp.tile([64, 7, 7], f32, tag=f"t2{b}", name=f"t2{b}")
                nc.vector.tensor_tensor(t1[:], zr4[:, :, 0, :, 0], zr4[:, :, 0, :, 1],
                                        op=Alu.add)
                nc.vector.tensor_tensor(t2[:], zr4[:, :, 1, :, 0], zr4[:, :, 1, :, 1],
                                        op=Alu.add)
                nc.vector.tensor_tensor(z9b[:, b, 1:8, 1:8], t1[:], t2[:], op=Alu.add)

            # ---- attention head (batched; feeds only the final combine) ----
            ps_a = pp_m.tile([32, 7, 7], f32, tag="psm")
            nc.tensor.matmul(ps_a[:], fc1l, ctx9[:, 1:8, 1:8], start=True, stop=True)
            a_sb = wp.tile([32, 7, 7], bf16, tag="a_sb")
            nc.scalar.activation(a_sb[:], ps_a[:], Act.Relu)
            a9 = wp.tile([32, 9, 9], bf16, tag="a9")
            nc.gpsimd.memset(a9[:], 0.0)
            nc.vector.tensor_copy(a9[:, 1:8, 1:8], a_sb[:])
            ps_att = pp_m.tile([8, 7, 7], f32, tag="psm")
            for d in range(9):
                dy, dx = d // 3, d % 3
                nc.tensor.matmul(ps_att[:], fc2l(d), a9[:, dy:dy + 7, dx:dx + 7],
                                 start=(d == 0), stop=(d == 8))
            att_sb = wp.tile([8, 7, 7], f32, tag="att_sb")
            nc.scalar.activation(att_sb[:], ps_att[:], Act.Identity, bias=fc2b, scale=1.0)
            for b in range(BL):
                nc.gpsimd.dma_start(
                    out=attflat[b].rearrange("(k q) -> k q", k=4),
                    in_=att_sb[b * 4:(b + 1) * 4].rearrange("k i j -> k (i j)"))

            # ---- regroup + LN + softmax + aw (both samples) ----
            v4 = wp.tile([49, BL, 4], f32, tag="v4")
            nc.gpsimd.dma_start(out=v4[:], in_=attflat[:].rearrange("b (p k) -> p b k", k=4))
            s1 = wp.tile([49, BL], f32, tag="s1")
            nc.vector.tensor_reduce(s1[:], v4[:], op=Alu.add, axis=mybir.AxisListType.X)
            sq = wp.tile([49, BL, 4], f32, tag="sq")
            nc.scalar.activation(sq[:], v4[:], Act.Square)
            s2 = wp.tile([49, BL], f32, tag="s2")
            nc.vector.tensor_reduce(s2[:], sq[:], op=Alu.add, axis=mybir.AxisListType.X)
            mu = wp.tile([49, BL], f32, tag="mu")
            nc.vector.tensor_scalar_mul(mu[:], s1[:], 0.25)
            mu2 = wp.tile([49, BL], f32, tag="mu2")
            nc.vector.tensor_mul(mu2[:], mu[:], mu[:])
            var = wp.tile([49, BL], f32, tag="var")
            nc.vector.scalar_tensor_tensor(var[:], s2[:], 0.25, mu2[:],
                                           op0=Alu.mult, op1=Alu.subtract)
            nc.vector.tensor_scalar_add(var[:], var[:], 1e-5)
            rec = wp.tile([49, BL], f32, tag="rec")
            nc.vector.reciprocal(rec[:], var[:])
            rstd = wp.tile([49, BL], f32, tag="rstd")
            nc.scalar.activation(rstd[:], rec[:], Act.Sqrt)
            y = wp.tile([49, BL, 4], f32, tag="y")
            nc.vector.tensor_sub(y[:], v4[:], mu[:].unsqueeze(2).broadcast_to([49, BL, 4]))
            nc.vector.tensor_mul(y[:], y[:], rstd[:].unsqueeze(2).broadcast_to([49, BL, 4]))
            nc.vector.tensor_mul(y[:], y[:], lng8.rearrange("p (b k) -> p b k", k=4))
            nc.vector.tensor_add(y[:], y[:], lnb8.rearrange("p (b k) -> p b k", k=4))
            z = wp.tile([49, BL, 4], f32, tag="z")
            nc.scalar.activation(z[:], y[:], Act.Exp, scale=1.0 / TEMP)
            ps_zs = pp_m.tile([1, BL * 4], f32, tag="psm")
            nc.tensor.matmul(ps_zs[:], ones49, z[:].rearrange("p b k -> p (b k)"),
                             start=True, stop=True)
            zrec = wp.tile([1, BL * 4], f32, tag="zrec")
            nc.vector.reciprocal(zrec[:], ps_zs[:])
            ps_rb = pp_m.tile([49, BL * 4], f32, tag="psm")
            nc.tensor.matmul(ps_rb[:], ones1x[:, 0:49], zrec[:], start=True, stop=True)
            attn = wp.tile([49, BL, 4], f32, tag="attn")
            nc.vector.tensor_mul(attn[:], z[:], ps_rb[:].rearrange("p (b k) -> p b k", k=4))
            junk = wp.tile([49, 4], f32, tag="junk")
            asums = wp.tile([49, BL * 4], f32, tag="asums")
            for b in range(BL):
                for s in range(4):
                    nc.vector.scalar_tensor_tensor(
                        junk[:], attn[:, b], 1.0, m4(s), op0=Alu.mult, op1=Alu.mult,
                        accum_out=asums[:, b * 4 + s:b * 4 + s + 1])
            ps_aw = pp_m.tile([1, BL * 4], f32, tag="psm")
            nc.tensor.matmul(ps_aw[:], ones49, asums[:], start=True, stop=True)
            aw_sb = wp.tile([1, BL * 4], f32, tag="aw_sb")
            nc.vector.tensor_copy(aw_sb[:], ps_aw[:])
            ps_awb = pp_m.tile([64, BL * 4], f32, tag="psm")
            nc.tensor.matmul(ps_awb[:], ones1x[:, 0:64], aw_sb[:], start=True, stop=True)
            awb = wp.tile([64, BL * 4], f32, tag="awb")
            nc.vector.tensor_copy(awb[:], ps_awb[:])

            # ================= final conv (batched) + combine =================
            ps_fc = pp_c.tile([64, 4, BL, 7, 7], f32, tag="ps_fc")
            for k in range(4):
                for d in range(9):
                    dy, dx = d // 3, d % 3
                    nc.tensor.matmul(ps_fc[:, k], wl(k, d),
                                     z9b[:, :, dy:dy + 7, dx:dx + 7],
                                     start=(d == 0), stop=(d == 8))
            for b in range(BL):
                eng = nc.vector
                acc = wp.tile([64, 7, 7], f32, tag=f"acc{b}", name=f"acc{b}")
                eng.tensor_scalar(acc[:], ps_fc[:, 0, b], awb[:, b * 4:b * 4 + 1], None,
                                  op0=Alu.mult)
                for k in range(1, 4):
                    eng.scalar_tensor_tensor(acc[:], ps_fc[:, k, b],
                                             awb[:, b * 4 + k:b * 4 + k + 1], acc[:],
                                             op0=Alu.mult, op1=Alu.add)
                aggb = wp.tile([64, 1], f32, tag=f"aggb{b}", name=f"aggb{b}")
                eng.tensor_scalar(aggb[:], biasT[0:64, 0:1], awb[:, b * 4:b * 4 + 1], None,
                                  op0=Alu.mult)
                for k in range(1, 4):
                    eng.scalar_tensor_tensor(aggb[:], biasT[0:64, k:k + 1],
                                             awb[:, b * 4 + k:b * 4 + k + 1], aggb[:],
                                             op0=Alu.mult, op1=Alu.add)
                out_sb = wp.tile([64, 7, 7], f32, tag=f"out_sb{b}", name=f"out_sb{b}")
                eng.tensor_scalar(out_sb[:], acc[:], aggb[:], None, op0=Alu.add)
                (nc.sync if b == 0 else nc.scalar).dma_start(out=out_h[b], in_=out_sb[:])

    nc.compile()
    return nc


def get_nc():
    if "nc" not in _CACHED:
        _CACHED["nc"] = build_nc()
    return _CACHED["nc"]


# --------------------------------------------------------------------------
# entry point
# --------------------------------------------------------------------------
def make_in_maps(inputs):
    import ml_dtypes
    x = np.ascontiguousarray(np.asarray(inputs["x"], np.float32))
    consts = make_consts(inputs)
    in_maps = [dict(**consts) for _ in range(NCORES)]
    # [b, c, h, w] -> [b, h, s, c, j] with w = 32*j + s
    xs5 = x.reshape(B, C, H, 7, 32).transpose(0, 2, 4, 1, 3).astype(ml_dtypes.bfloat16)
    xg = x.astype(ml_dtypes.bfloat16)
    for c in range(NCORES):
        in_maps[c]["xs5"] = np.ascontiguousarray(xs5[c * BL:(c + 1) * BL])
        in_maps[c]["xg"] = np.ascontiguousarray(xg[c * BL:(c + 1) * BL])
    return in_maps


def kernel(**inputs):
    from concourse.bass_utils import run_bass_kernel_spmd

    nc = get_nc()
    in_maps = make_in_maps(inputs)
    res = run_bass_kernel_spmd(nc, in_maps, list(range(NCORES)))
    return np.concatenate([m["out"] for m in res.results], axis=0)


# revision 27
# speedup vs baseline: 2.8113x; 2.8113x over previous
"""Trainium2 Bass kernel for nn_CNN2D_37873021616665 (dense_cnn).

Data-parallel over batch: 16 samples -> 8 cores x 2 samples. Per core:
  1. 32x32 mean-pool on the PE: host layout xs5[b,h,s,c,j] (w = 32j+s) makes
     every moving operand a contiguous [h, (c j)] slab; 64 matmuls accumulate
     straight into ctx [7i, (c j)] in one PSUM bank per sample.
  2. asymmetric DMA rings: SP carries only b0's h<128 half, ACT the rest, so
     b0's context is ready ~3x earlier and its whole roi-head -> box ->
     gather -> interp chain hides under b1's pooling stream (HWDGE rings are
     FIFO, so b0's gathers would otherwise queue behind the stream).
  3. per-sample roi/box path: b1's ctx lands on partitions 64:127 and uses
     the second copy of the block-diagonal weights; boxes do not depend on
     the attention branch at all.
  4. attention head batched over samples on (b, c) partitions; LN/softmax/aw
     (with the torch-.view regroup via a DRAM round-trip) overlaps gathers.
  5. bilinear combine with clamp/validity/0.25-subsample folded into
     weights; final 3x3 conv batched over samples (36 bf16 matmuls, N=98),
     combined with the attention-derived scalars by linearity.
"""
import numpy as np

B = 16
NCORES = 8
BL = B // NCORES          # samples per core
C = 64
H = W = 224
POOL = 7
NK = 4
TEMP = 5.0

_CACHED = {}


# --------------------------------------------------------------------------
# host-side constant packing (pure layout; replicated to all cores)
# --------------------------------------------------------------------------
def make_consts(inputs):
    import ml_dtypes
    f32 = np.float32
    bf16 = ml_dtypes.bfloat16
    fc1_w = np.asarray(inputs["fc1_w"], f32)
    fc2_w = np.asarray(inputs["fc2_w"], f32)
    fc2_b = np.asarray(inputs["fc2_b"], f32)
    ln_g = np.asarray(inputs["ln_g"], f32)
    ln_b = np.asarray(inputs["ln_b"], f32)
    roi_w1 = np.asarray(inputs["roi_w1"], f32)
    roi_w2 = np.asarray(inputs["roi_w2"], f32)
    roi_b2 = np.asarray(inputs["roi_b2"], f32)
    weight = np.asarray(inputs["weight"], f32)
    bias = np.asarray(inputs["bias"], f32)

    k128 = np.zeros((128, 7), f32)
    for h in range(128):
        k128[h, h // 32] = 1.0 / 1024.0
    k96 = np.zeros((96, 7), f32)
    for h in range(96):
        k96[h, 4 + h // 32] = 1.0 / 1024.0
    k128 = k128.astype(bf16)        # 1/1024 is exact in bf16
    k96 = k96.astype(bf16)

    def blk(m):  # [p, q] -> [2p, 2q] block diagonal
        p, q = m.shape
        out = np.zeros((2 * p, 2 * q), f32)
        out[:p, :q] = m
        out[p:, q:] = m
        return out

    # kbd [128, 2624] bf16: fc1 blkdiag | roi1 blkdiag x9 | conv-w [64,64] x36
    kbd = np.zeros((128, 2624), f32)
    kbd[:, 0:32] = blk(fc1_w[:, :, 0, 0].T)
    for d in range(9):
        dy, dx = d // 3, d % 3
        kbd[:, 32 + d * 32:32 + (d + 1) * 32] = blk(roi_w1[:, :, dy, dx].T)
    for k in range(4):
        for d in range(9):
            dy, dx = d // 3, d % 3
            kbd[0:64, 320 + (k * 9 + d) * 64:320 + (k * 9 + d + 1) * 64] = \
                weight[k, :, :, dy, dx].T
    kbd = kbd.astype(bf16)

    # k32 [32, 80] bf16: fc2 blkdiag x9 | roi_w2/49 (single copy, rows 0:16)
    k32 = np.zeros((32, 80), f32)
    for d in range(9):
        dy, dx = d // 3, d % 3
        k32[:, d * 8:(d + 1) * 8] = blk(fc2_w[:, :, dy, dx].T)
    k32[0:16, 72:76] = (roi_w2[:4, :, 0, 0] / 49.0).T
    k32 = k32.astype(bf16)

    k49 = np.zeros((49, 33), f32)
    k49[:, 0:8] = np.tile(ln_g, (49, 2))
    k49[:, 8:16] = np.tile(ln_b, (49, 2))
    t = np.arange(196).reshape(49, 4)
    for s in range(4):
        k49[:, 16 + s * 4:16 + (s + 1) * 4] = ((t >= 49 * s) & (t < 49 * (s + 1)))
    k49[:, 32] = 1.0

    kbt = np.tile(bias.T, (2, 1)).astype(f32)     # [128, 4] (only 0:64 used)

    k8 = np.zeros((8, 2), f32)
    k8[:, 0] = np.tile(fc2_b, 2)
    k8[:, 1] = np.tile(roi_b2[:4] * 224.0, 2)

    k7 = np.eye(8, dtype=f32)

    k1 = np.zeros((1, 142), f32)
    off = (np.arange(7, dtype=f32)[:, None] + (np.arange(2, dtype=f32)[None, :] + 0.5) / 2)
    k1[0, 0:14] = off.reshape(-1)
    k1[0, 14:142] = 1.0

    return {"k128": k128, "k96": k96, "kbd": kbd, "k32": k32, "k49": k49,
            "kbt": kbt, "k8": k8, "k7": k7, "k1": k1}


# --------------------------------------------------------------------------
# device program
# --------------------------------------------------------------------------
def build_nc(repeat=1):
    import concourse.bass as bass
    import concourse.bacc as bacc
    import concourse.mybir as mybir
    import concourse.tile as tile
    from concourse.bass import ds

    f32 = mybir.dt.float32
    i32 = mybir.dt.int32
    bf16 = mybir.dt.bfloat16
    Alu = mybir.AluOpType
    Act = mybir.ActivationFunctionType
    ET = mybir.EngineType

    nc = bacc.Bacc("TRN2", target_bir_lowering=False)

    xs5_h = nc.dram_tensor("xs5", [BL, H, 32, C, 7], bf16, kind="ExternalInput")
    xg_h = nc.dram_tensor("xg", [BL, C, H, W], bf16, kind="ExternalInput")
    k128_h = nc.dram_tensor("k128", [128, 7], bf16, kind="ExternalInput")
    k96_h = nc.dram_tensor("k96", [96, 7], bf16, kind="ExternalInput")
    kbd_h = nc.dram_tensor("kbd", [128, 2624], bf16, kind="ExternalInput")
    k32_h = nc.dram_tensor("k32", [32, 80], bf16, kind="ExternalInput")
    k49_h = nc.dram_tensor("k49", [49, 33], f32, kind="ExternalInput")
    kbt_h = nc.dram_tensor("kbt", [128, 4], f32, kind="ExternalInput")
    k8_h = nc.dram_tensor("k8", [8, 2], f32, kind="ExternalInput")
    k7_h = nc.dram_tensor("k7", [8, 8], f32, kind="ExternalInput")
    k1_h = nc.dram_tensor("k1", [1, 142], f32, kind="ExternalInput")
    out_h = nc.dram_tensor("out", [BL, C, POOL, POOL], f32, kind="ExternalOutput")

    xg_flat = xg_h[:].rearrange("b c h w -> b c (h w)")

    from contextlib import ExitStack

    with tile.TileContext(nc) as tc, ExitStack() as est:
        cp = est.enter_context(tc.tile_pool(name="const", bufs=1))
        xp = est.enter_context(tc.tile_pool(name="xtiles", bufs=4))
        wp = est.enter_context(tc.tile_pool(name="work", bufs=2))
        gp = est.enter_context(tc.tile_pool(name="gather", bufs=1))
        pp_acc = est.enter_context(tc.tile_pool(name="ps_acc", bufs=2, space="PSUM"))
        pp_m = est.enter_context(tc.tile_pool(name="ps_misc", bufs=2, space="PSUM"))
        pp_c = est.enter_context(tc.tile_pool(name="ps_conv", bufs=1, space="PSUM"))
        dp = est.enter_context(tc.tile_pool(name="dscratch", bufs=1, space="DRAM"))

        # ---- load constants (gpsimd ring: keeps SP/ACT free for x) ----
        kc = {}
        for name, hdl, shp, dt_ in (("k128", k128_h, [128, 7], bf16),
                                    ("k96", k96_h, [96, 7], bf16),
                                    ("kbd", kbd_h, [128, 2624], bf16),
                                    ("k32", k32_h, [32, 80], bf16),
                                    ("k49", k49_h, [49, 33], f32),
                                    ("kbt", kbt_h, [128, 4], f32),
                                    ("k8", k8_h, [8, 2], f32),
                                    ("k7", k7_h, [8, 8], f32),
                                    ("k1", k1_h, [1, 142], f32)):
            t = cp.tile(shp, dt_, tag=name)
            nc.gpsimd.dma_start(out=t[:], in_=hdl[:])
            kc[name] = t
        fc1l = kc["kbd"][:, 0:32]
        # roi1 block for sample b: b0 -> top-left, b1 -> bottom-right copy
        roi1l = lambda b, d: kc["kbd"][b * 64:(b + 1) * 64,
                                       32 + d * 32 + b * 16:32 + d * 32 + b * 16 + 16]
        wl = lambda k, d: kc["kbd"][0:64, 320 + (k * 9 + d) * 64:320 + (k * 9 + d + 1) * 64]
        fc2l = lambda d: kc["k32"][:, d * 8:(d + 1) * 8]
        w2l = kc["k32"][0:16, 72:76]
        lng8 = kc["k49"][:, 0:8]
        lnb8 = kc["k49"][:, 8:16]
        m4 = lambda s: kc["k49"][:, 16 + s * 4:16 + (s + 1) * 4]
        ones49 = kc["k49"][:, 32:33]
        biasT = kc["kbt"][:, :]
        fc2b = kc["k8"][:, 0:1]
        b224 = lambda b: kc["k8"][0:4, 1:2]
        id8 = kc["k7"][:]
        id7 = kc["k7"][0:7, 0:7]
        id4 = kc["k7"][0:4, 0:4]
        offv = kc["k1"][:, 0:14]
        ones1x = kc["k1"][:, 14:142]          # [1, 128] of ones

        for _rep in range(repeat):
            attflat = dp.tile([BL, 196], f32)
            NCK = 4

            # ---- issue every pooling chunk DMA up front (queues are
            # per-engine): SP ring = b0 h<128 only, ACT ring = the rest, so
            # SP drains early and b0's gathers aren't stuck behind the stream
            tiles = {}
            for b in range(BL):
                eng_a = nc.sync if b == 0 else nc.scalar
                for ck in range(NCK):
                    ta = xp.tile([128, 8, 448], bf16, tag=f"TA{b}", name=f"TA{b}_{ck}")
                    tb = xp.tile([96, 8, 448], bf16, tag=f"TB{b}", name=f"TB{b}_{ck}")
                    eng_a.dma_start(
                        out=ta[:],
                        in_=xs5_h[b, 0:128, ck * 8:(ck + 1) * 8]
                        .rearrange("p s c j -> p s (c j)"))
                    nc.scalar.dma_start(
                        out=tb[:],
                        in_=xs5_h[b, 128:224, ck * 8:(ck + 1) * 8]
                        .rearrange("p s c j -> p s (c j)"))
                    tiles[(b, ck)] = (ta, tb)

            # shared tiles for the per-sample chains
            ps_ctxT = pp_m.tile([128, 7, 7], f32, tag="psT")
            ctx9 = wp.tile([128, 9, 9], bf16, tag="ctx9")
            nc.gpsimd.memset(ctx9[:], 0.0)
            z9b = wp.tile([64, BL, 9, 9], bf16, tag="z9b")
            nc.gpsimd.memset(z9b[:], 0.0)
            ps_pool_l = []
            wball_l = []

            # ========== per-sample: pool -> ctx -> roi -> box -> gather ====
            # (b0's whole chain is issued BEFORE b1's pooling matmuls: the PE
            # queue is strict FIFO, so anything issued after b1's pool MMs
            # would stall behind the DMA-paced stream)
            for b in range(BL):
                ps_pool = pp_acc.tile([7, 448], f32, tag="poolacc")
                for ck in range(NCK):
                    ta, tb = tiles[(b, ck)]
                    for sl in range(8):
                        nc.tensor.matmul(ps_pool[:], kc["k128"][:], ta[:, sl, :],
                                         start=(ck == 0 and sl == 0), stop=False)
                    for sl in range(8):
                        nc.tensor.matmul(ps_pool[:], kc["k96"][:], tb[:, sl, :],
                                         start=False, stop=(ck == NCK - 1 and sl == 7))
                ps_pool_l.append(ps_pool)
                pl, ph = b * 64, (b + 1) * 64
                ctx_sbb = wp.tile([7, 64, 7], f32, tag=f"ctx{b}", name=f"ctx{b}")
                nc.vector.tensor_copy(ctx_sbb[:], ps_pool[:].rearrange(
                    "p (c j) -> p c j", j=7))
                for j in range(7):
                    # plain matmul against the identity == transpose, but is
                    # allowed to land on PSUM base-partition 64 (b1's half)
                    nc.tensor.matmul(ps_ctxT[pl:ph, :, j], ctx_sbb[:, :, j], id7,
                                     start=True, stop=True)
                nc.vector.tensor_copy(ctx9[pl:ph, 1:8, 1:8], ps_ctxT[pl:ph])

                # roi head (second blkdiag copy for b1; all outputs at base 0)
                ps_r = pp_m.tile([16, 7, 7], f32, tag="psm")
                for d in range(9):
                    dy, dx = d // 3, d % 3
                    nc.tensor.matmul(ps_r[:], roi1l(b, d),
                                     ctx9[pl:ph, dy:dy + 7, dx:dx + 7],
                                     start=(d == 0), stop=(d == 8))
                r_sb = wp.tile([16, 7, 7], f32, tag=f"r_sb{b}", name=f"r_sb{b}")
                rsum = wp.tile([16, 1], f32, tag=f"rsum{b}", name=f"rsum{b}")
                nc.scalar.activation(r_sb[:], ps_r[:], Act.Relu, accum_out=rsum[:])
                rsum_b = wp.tile([16, 1], bf16, tag=f"rsumb{b}", name=f"rsumb{b}")
                nc.vector.tensor_copy(rsum_b[:], rsum[:])
                ps_bb = pp_m.tile([4, 1], f32, tag="psm")
                nc.tensor.matmul(ps_bb[:], w2l, rsum_b[:], start=True, stop=True)
                bb_sb = wp.tile([4, 1], f32, tag=f"bb{b}", name=f"bb{b}")
                nc.scalar.activation(bb_sb[:], ps_bb[:], Act.Identity,
                                     bias=b224(b), scale=224.0)
                ps_bx = pp_m.tile([1, 4], f32, tag="psm")
                nc.tensor.transpose(ps_bx[:], bb_sb[:], id4)
                bx = wp.tile([1, 4], f32, tag=f"bx{b}", name=f"bx{b}")
                nc.vector.tensor_copy(bx[:], ps_bx[:])

                # box math [1, {x,y}, 14]
                bm = nc.vector
                d2 = wp.tile([1, 2], f32, tag=f"d2{b}", name=f"d2{b}")
                bm.tensor_sub(d2[:], bx[:, 2:4], bx[:, 0:2])
                bm.tensor_scalar_max(d2[:], d2[:], 1.0)
                bm.tensor_scalar_mul(d2[:], d2[:], 1.0 / 7.0)
                cs = wp.tile([1, 2, 14], f32, tag=f"cs{b}", name=f"cs{b}")
                for ax in range(2):
                    bm.scalar_tensor_tensor(
                        cs[:, ax], offv, d2[:, ax:ax + 1],
                        bx[:, ax:ax + 1].broadcast_to([1, 14]),
                        op0=Alu.mult, op1=Alu.add)
                va = wp.tile([1, 2, 14], f32, tag=f"va{b}", name=f"va{b}")
                vb = wp.tile([1, 2, 14], f32, tag=f"vb{b}", name=f"vb{b}")
                bm.tensor_scalar(va[:], cs[:], -1.0, None, op0=Alu.is_ge)
                bm.tensor_scalar(vb[:], cs[:], 224.0, None, op0=Alu.is_le)
                bm.tensor_mul(va[:], va[:], vb[:])
                # fold the 2x2-subsample 1/4 into validity (1/2 per axis)
                bm.tensor_scalar_mul(va[:], va[:], 0.5)
                cc = wp.tile([1, 2, 14], f32, tag=f"cc{b}", name=f"cc{b}")
                bm.tensor_scalar(cc[:], cs[:], 0.0, 223.0, op0=Alu.max, op1=Alu.min)
                iraw = wp.tile([1, 2, 14], i32, tag=f"iraw{b}", name=f"iraw{b}")
                bm.tensor_copy(iraw[:], cc[:])
                c0 = wp.tile([1, 2, 14], f32, tag=f"c0{b}", name=f"c0{b}")
                bm.tensor_copy(c0[:], iraw[:])
                cgt = wp.tile([1, 2, 14], f32, tag=f"cgt{b}", name=f"cgt{b}")
                bm.tensor_tensor(cgt[:], c0[:], cc[:], op=Alu.is_gt)
                bm.tensor_sub(c0[:], c0[:], cgt[:])
                bm.tensor_scalar_min(c0[:], c0[:], 222.0)
                fr = wp.tile([1, 2, 14], f32, tag=f"fr{b}", name=f"fr{b}")
                bm.tensor_sub(fr[:], cc[:], c0[:])
                tw = wp.tile([1, 2, 14], f32, tag=f"tw{b}", name=f"tw{b}")
                bm.tensor_scalar(tw[:], fr[:], -1.0, 1.0, op0=Alu.mult, op1=Alu.add)
                wb_sb = wp.tile([1, 2, 14, 2], f32, tag=f"wbs{b}", name=f"wbs{b}")
                bm.tensor_mul(wb_sb[:, :, :, 0], tw[:], va[:])
                bm.tensor_mul(wb_sb[:, :, :, 1], fr[:], va[:])
                ioff = wp.tile([1, 2, 14], f32, tag=f"ioff{b}", name=f"ioff{b}")
                bm.tensor_copy(ioff[:, 0], c0[:, 0])
                bm.tensor_scalar_mul(ioff[:, 1], c0[:, 1], 224.0)
                ioffi = wp.tile([1, 2, 14], i32, tag=f"ioffi{b}", name=f"ioffi{b}")
                bm.tensor_copy(ioffi[:], ioff[:])

                ps_wb = pp_m.tile([64, 56], f32, tag="psm")
                nc.tensor.matmul(ps_wb[:], ones1x[:, 0:64],
                                 wb_sb[:].rearrange("p t j a -> p (t j a)"),
                                 start=True, stop=True)
                wball = wp.tile([64, 2, 14, 2], f32, tag=f"wball{b}", name=f"wball{b}")
                nc.scalar.activation(wball[:].rearrange("p t j a -> p (t j a)"),
                                     ps_wb[:], Act.Identity)
                wball_l.append(wball)

                # gathers: both samples on the SP ring (idle once b0's half is in)
                _, yvals = nc.values_load_multi_w_load_instructions(
                    ioffi[:, 1, :], engines=[ET.SP],
                    min_val=0, max_val=49728, skip_runtime_bounds_check=True)
                g2 = gp.tile([64, 14, 2, 224], bf16, tag=f"g2{b}", name=f"g2{b}")
                for i in range(14):
                    nc.sync.dma_start(out=g2[:, i], in_=xg_flat[b, :, ds(yvals[i], 448)]
                                      .rearrange("c (r w) -> c r w", r=2))
                _, xvals = nc.values_load_multi_w_load_instructions(
                    ioffi[:, 0, :], engines=[ET.DVE],
                    min_val=0, max_val=222, skip_runtime_bounds_check=True)
                # x-interp: per-j pair * (wA, wB)
                m1 = wp.tile([64, 14, 2, 14, 2], f32, tag=f"m1{b}", name=f"m1{b}")
                for j in range(14):
                    nc.vector.tensor_tensor(
                        m1[:, :, :, j, :], g2[:, :, :, ds(xvals[j], 2)],
                        wball[:, 0, j].unsqueeze(1).unsqueeze(1)
                        .broadcast_to([64, 14, 2, 2]), op=Alu.mult)
                zc4 = wp.tile([64, 14, 2, 14], f32, tag=f"zc4{b}", name=f"zc4{b}")
                nc.vector.tensor_tensor(zc4[:], m1[:, :, :, :, 0], m1[:, :, :, :, 1],
                                        op=Alu.add)
                m2 = wp.tile([64, 14, 2, 14], f32, tag=f"m2{b}", name=f"m2{b}")
                nc.vector.tensor_tensor(m2[:], zc4[:],
                                        wball[:, 1].unsqueeze(3)
                                        .broadcast_to([64, 14, 2, 14]), op=Alu.mult)
                zr = wp.tile([64, 14, 14], f32, tag=f"zr{b}", name=f"zr{b}")
                nc.vector.tensor_tensor(zr[:], m2[:, :, 0, :], m2[:, :, 1, :], op=Alu.add)
                zr4 = zr[:].rearrange("p (i a) (j e) -> p i a j e", a=2, e=2)
                t1 = wp.tile([64, 7, 7], f32, tag=f"t1{b}", name=f"t1{b}")
                t2 = wp.tile([64, 7, 7], f32, tag=f"t2{b}", name=f"t2{b}")
                nc.vector.tensor_tensor(t1[:], zr4[:, :, 0, :, 0], zr4[:, :, 0, :, 1],
                                        op=Alu.add)
                nc.vector.tensor_tensor(t2[:], zr4[:, :, 1, :, 0], zr4[:, :, 1, :, 1],
                                        op=Alu.add)
                nc.vector.tensor_tensor(z9b[:, b, 1:8, 1:8], t1[:], t2[:], op=Alu.add)

            # ---- attention head (batched; feeds only the final combine) ----
            ps_a = pp_m.tile([32, 7, 7], f32, tag="psm")
            nc.tensor.matmul(ps_a[:], fc1l, ctx9[:, 1:8, 1:8], start=True, stop=True)
            a_sb = wp.tile([32, 7, 7], bf16, tag="a_sb")
            nc.scalar.activation(a_sb[:], ps_a[:], Act.Relu)
            a9 = wp.tile([32, 9, 9], bf16, tag="a9")
            nc.gpsimd.memset(a9[:], 0.0)
            nc.vector.tensor_copy(a9[:, 1:8, 1:8], a_sb[:])
            ps_att = pp_m.tile([8, 7, 7], f32, tag="psm")
            for d in range(9):
                dy, dx = d // 3, d % 3
                nc.tensor.matmul(ps_att[:], fc2l(d), a9[:, dy:dy + 7, dx:dx + 7],
                                 start=(d == 0), stop=(d == 8))
            att_sb = wp.tile([8, 7, 7], f32, tag="att_sb")
            nc.scalar.activation(att_sb[:], ps_att[:], Act.Identity, bias=fc2b, scale=1.0)
            for b in range(BL):
                nc.gpsimd.dma_start(
                    out=attflat[b].rearrange("(k q) -> k q", k=4),
                    in_=att_sb[b * 4:(b + 1) * 4].rearrange("k i j -> k (i j)"))

            # ---- regroup + LN + softmax + aw (both samples) ----
            v4 = wp.tile([49, BL, 4], f32, tag="v4")
            nc.gpsimd.dma_start(out=v4[:], in_=attflat[:].rearrange("b (p k) -> p b k", k=4))
            s1 = wp.tile([49, BL], f32, tag="s1")
            nc.vector.tensor_reduce(s1[:], v4[:], op=Alu.add, axis=mybir.AxisListType.X)
            sq = wp.tile([49, BL, 4], f32, tag="sq")
            nc.scalar.activation(sq[:], v4[:], Act.Square)
            s2 = wp.tile([49, BL], f32, tag="s2")
            nc.vector.tensor_reduce(s2[:], sq[:], op=Alu.add, axis=mybir.AxisListType.X)
            mu = wp.tile([49, BL], f32, tag="mu")
            nc.vector.tensor_scalar_mul(mu[:], s1[:], 0.25)
            mu2 = wp.tile([49, BL], f32, tag="mu2")
            nc.vector.tensor_mul(mu2[:], mu[:], mu[:])
            var = wp.tile([49, BL], f32, tag="var")
            nc.vector.scalar_tensor_tensor(var[:], s2[:], 0.25, mu2[:],
                                           op0=Alu.mult, op1=Alu.subtract)
            nc.vector.tensor_scalar_add(var[:], var[:], 1e-5)
            rec = wp.tile([49, BL], f32, tag="rec")
            nc.vector.reciprocal(rec[:], var[:])
            rstd = wp.tile([49, BL], f32, tag="rstd")
            nc.scalar.activation(rstd[:], rec[:], Act.Sqrt)
            y = wp.tile([49, BL, 4], f32, tag="y")
            nc.vector.tensor_sub(y[:], v4[:], mu[:].unsqueeze(2).broadcast_to([49, BL, 4]))
            nc.vector.tensor_mul(y[:], y[:], rstd[:].unsqueeze(2).broadcast_to([49, BL, 4]))
            nc.vector.tensor_mul(y[:], y[:], lng8.rearrange("p (b k) -> p b k", k=4))
            nc.vector.tensor_add(y[:], y[:], lnb8.rearrange("p (b k) -> p b k", k=4))
            z = wp.tile([49, BL, 4], f32, tag="z")
            nc.scalar.activation(z[:], y[:], Act.Exp, scale=1.0 / TEMP)
            ps_zs = pp_m.tile([1, BL * 4], f32, tag="psm")
            nc.tensor.matmul(ps_zs[:], ones49, z[:].rearrange("p b k -> p (b k)"),
                             start=True, stop=True)
            zrec = wp.tile([1, BL * 4], f32, tag="zrec")
            nc.vector.reciprocal(zrec[:], ps_zs[:])
            ps_rb = pp_m.tile([49, BL * 4], f32, tag="psm")
            nc.tensor.matmul(ps_rb[:], ones1x[:, 0:49], zrec[:], start=True, stop=True)
            attn = wp.tile([49, BL, 4], f32, tag="attn")
            nc.vector.tensor_mul(attn[:], z[:], ps_rb[:].rearrange("p (b k) -> p b k", k=4))
            junk = wp.tile([49, 4], f32, tag="junk")
            asums = wp.tile([49, BL * 4], f32, tag="asums")
            for b in range(BL):
                for s in range(4):
                    nc.vector.scalar_tensor_tensor(
                        junk[:], attn[:, b], 1.0, m4(s), op0=Alu.mult, op1=Alu.mult,
                        accum_out=asums[:, b * 4 + s:b * 4 + s + 1])
            ps_aw = pp_m.tile([1, BL * 4], f32, tag="psm")
            nc.tensor.matmul(ps_aw[:], ones49, asums[:], start=True, stop=True)
            aw_sb = wp.tile([1, BL * 4], f32, tag="aw_sb")
            nc.vector.tensor_copy(aw_sb[:], ps_aw[:])
            ps_awb = pp_m.tile([64, BL * 4], f32, tag="psm")
            nc.tensor.matmul(ps_awb[:], ones1x[:, 0:64], aw_sb[:], start=True, stop=True)
            awb = wp.tile([64, BL * 4], f32, tag="awb")
            nc.vector.tensor_copy(awb[:], ps_awb[:])

            # ================= final conv (batched) + combine =================
            ps_fc = pp_c.tile([64, 4, BL, 7, 7], f32, tag="ps_fc")
            for k in range(4):
                for d in range(9):
                    dy, dx = d // 3, d % 3
                    nc.tensor.matmul(ps_fc[:, k], wl(k, d),
                                     z9b[:, :, dy:dy + 7, dx:dx + 7],
                                     start=(d == 0), stop=(d == 8))
            for b in range(BL):
                eng = nc.vector
                acc = wp.tile([64, 7, 7], f32, tag=f"acc{b}", name=f"acc{b}")
                eng.tensor_scalar(acc[:], ps_fc[:, 0, b], awb[:, b * 4:b * 4 + 1], None,
                                  op0=Alu.mult)
                for k in range(1, 4):
                    eng.scalar_tensor_tensor(acc[:], ps_fc[:, k, b],
                                             awb[:, b * 4 + k:b * 4 + k + 1], acc[:],
                                             op0=Alu.mult, op1=Alu.add)
                aggb = wp.tile([64, 1], f32, tag=f"aggb{b}", name=f"aggb{b}")
                eng.tensor_scalar(aggb[:], biasT[0:64, 0:1], awb[:, b * 4:b * 4 + 1], None,
                                  op0=Alu.mult)
                for k in range(1, 4):
                    eng.scalar_tensor_tensor(aggb[:], biasT[0:64, k:k + 1],
                                             awb[:, b * 4 + k:b * 4 + k + 1], aggb[:],
                                             op0=Alu.mult, op1=Alu.add)
                out_sb = wp.tile([64, 7, 7], f32, tag=f"out_sb{b}", name=f"out_sb{b}")
                eng.tensor_scalar(out_sb[:], acc[:], aggb[:], None, op0=Alu.add)
                (nc.sync if b == 0 else nc.scalar).dma_start(out=out_h[b], in_=out_sb[:])

    nc.compile()
    return nc


def get_nc():
    if "nc" not in _CACHED:
        _CACHED["nc"] = build_nc()
    return _CACHED["nc"]


# --------------------------------------------------------------------------
# entry point
# --------------------------------------------------------------------------
def make_in_maps(inputs):
    import ml_dtypes
    x = np.ascontiguousarray(np.asarray(inputs["x"], np.float32))
    consts = make_consts(inputs)
    in_maps = [dict(**consts) for _ in range(NCORES)]
    # [b, c, h, w] -> [b, h, s, c, j] with w = 32*j + s
    xs5 = x.reshape(B, C, H, 7, 32).transpose(0, 2, 4, 1, 3).astype(ml_dtypes.bfloat16)
    xg = x.astype(ml_dtypes.bfloat16)
    for c in range(NCORES):
        in_maps[c]["xs5"] = np.ascontiguousarray(xs5[c * BL:(c + 1) * BL])
        in_maps[c]["xg"] = np.ascontiguousarray(xg[c * BL:(c + 1) * BL])
    return in_maps


def kernel(**inputs):
    from concourse.bass_utils import run_bass_kernel_spmd

    nc = get_nc()
    in_maps = make_in_maps(inputs)
    res = run_bass_kernel_spmd(nc, in_maps, list(range(NCORES)))
    return np.concatenate([m["out"] for m in res.results], axis=0)
